# revision 12
# baseline (speedup 1.0000x reference)
"""Trainium2 Bass kernel for nn_ChannelFusedCrossAttn.

Reference computation (per batch b, with N = H*W = 4096 spatial positions):
    ctx  = LeakyReLU_0.1(Wf @ context_fused + bf)        # [128, N]
    q    = Wq @ x + bq                                   # [32, N]
    k    = Wk @ ctx + bk                                 # [32, N]
    v    = Wv @ ctx + bv                                 # [256, N]
    attn = softmax(q^T k / sqrt(32), axis=keys)          # [N, N]
    out  = gamma * (Wo @ (v @ attn^T) + bo) + x

Sharding: 8 cores = 4 batches x 2 query-halves of 2048 positions each.
Each core computes ctx/k/v for the full key range of its batch (duplicated
across the pair) plus attention + output projection for its query half.

Device algorithm (per core, n = its 2048 query positions, m = 4096 keys):
  - scores are computed TRANSPOSED (scoreT[m-chunk, n]) so softmax's key-dim
    reduction and the attn@v contraction both keep m on partitions; the
    unnormalized exp() is used directly (scores here are ~N(0, 0.03), so no
    max-subtraction is needed) and the 1/rowsum normalization is applied after
    the v-contraction (division by a per-n scalar commutes with channel
    matmuls).
  - v is built transposed (vT[m, c] = ctx[:,m]^T @ Wv^T) so it can be the
    stationary matmul operand against E[m, n] without any transposes.
  - rowsum S[n] = sum_m E[m, n] rides the tensor engine: column-tiled
    [128,32]-of-ones matmuls (4 concurrent positions) accumulate 32x-replicated
    partial sums which a 1/32-scaled ones matmul then reduces+broadcasts.
  - biases: bq/bk/bf are applied on-chip via per-partition activation bias;
    bv/bo/gamma are folded on the host (gamma*Wo, gamma*(Wo@bv + bo)).
"""

import numpy as np
from contextlib import ExitStack

import concourse.bass as bass
import concourse.bacc as bacc
import concourse.tile as tile
from concourse import mybir
from concourse import bass_utils

F32 = mybir.dt.float32
BF16 = mybir.dt.bfloat16
FP8 = mybir.dt.float8e4
NP_BF16 = mybir.dt.np(BF16)
AF = mybir.ActivationFunctionType
ALU = mybir.AluOpType

# Problem shape (hardcoded per contest contract).
B = 4
Q_CH = 256
KV_CH = 128
NUM_CTX = 4
QK_DIM = 32
H = W = 64
N = H * W            # 4096 keys per batch
N_CORES = 8
NQ = 2048            # query positions per core (N * B / N_CORES)
SCALE = float(QK_DIM) ** -0.5

NT = 512             # n-tile (query) width for the attention inner loop
N_NT = NQ // NT      # 4
JG = 4               # score row-tile group size (concurrent PE row groups)
N_JG = (N // 128) // JG  # 8 j-groups of 4 key-chunks of 128


def _emit(nc, tc, ctx, d):
    """Emit the per-core program. `d` maps dram tensor name -> AP."""
    pool = ctx.enter_context(tc.tile_pool(name="sb", bufs=1))
    psum = ctx.enter_context(tc.tile_pool(name="ps", bufs=1, space="PSUM"))

    # ---- input streams first on the sync HWDGE ring (ctxin quarters so the
    # conv can start early), weights as two packed blobs on the scalar ring ----
    ctxin_sb = []
    for dd in range(NUM_CTX):
        t = pool.tile([128, N], BF16, name=f"ctxin{dd}", tag=f"ctxin{dd}")
        ctxin_sb.append(t)
    for hh in range(4):
        qsl_in = bass.ts(hh, N // 4)
        for dd in range(NUM_CTX):
            nc.sync.dma_start(ctxin_sb[dd][:, qsl_in],
                              d["ctxin"][dd * 128:(dd + 1) * 128, qsl_in])
        if hh == 0:
            x_sb = []
            for mm in range(2):
                t = pool.tile([128, NQ], F32, name=f"x{mm}", tag=f"x{mm}")
                nc.sync.dma_start(t[:], d["xin"][mm * 128:(mm + 1) * 128, :])
                x_sb.append(t)

    wb16 = pool.tile([128, 1408], BF16, tag="wb16")
    nc.scalar.dma_start(wb16[:], d["wblob16"][:, :])
    wb32 = pool.tile([128, 261], F32, tag="wb32")
    nc.scalar.dma_start(wb32[:], d["wblob32"][:, :])
    wf_sb = [wb16[:, dd * 128:(dd + 1) * 128] for dd in range(NUM_CTX)]
    wk_sb = wb16[:, 512:640]
    wv_sb = wb16[:, 640:896]
    wo_sb = [wb16[:, 896 + kk * 256:896 + (kk + 1) * 256] for kk in range(2)]
    wq_sb = [wb32[:, mm * 128:(mm + 1) * 128] for mm in range(2)]
    bf_sb = wb32[:, 256:257]
    bk_sb = wb32[:, 257:258]
    bq_sb = wb32[:, 258:259]
    gbo_sb = [wb32[:, 259 + mm:260 + mm] for mm in range(2)]

    ones32 = pool.tile([128, 32], FP8, tag="ones32")
    nc.gpsimd.memset(ones32[:], 1.0)
    ones_bc = pool.tile([128, 128], BF16, tag="ones_bc")
    nc.gpsimd.memset(ones_bc[:], 1.0 / 32.0)

    ctx_sb = pool.tile([128, N], BF16, tag="ctx")     # fused context, post-LeakyReLU
    kr_sb = pool.tile([128, N], BF16, tag="kr")       # k, 4x-replicated on partitions
    qr_sb = pool.tile([128, NQ], BF16, tag="qr")      # q, 4x-replicated on partitions
    # vT in fp8, pair-interleaved for DoubleRow: offset = t*512 + cc*256 + i*128 + c
    # (t = key-chunk pair, i = pair member, cc = channel chunk, c = channel)
    vt_sb = pool.tile([128, 32 * 256], FP8, tag="vt")
    out_sb = [pool.tile([128, NQ], F32, name=f"o{mm}", tag=f"o{mm}") for mm in range(2)]

    # ---- attention with all producer phases software-pipelined into nt=0:
    # per key-group g, nt0 emits conv(mt=g) -> k(mt=g) -> q(qt=g<4) -> vT(j in g)
    # ahead of that group's scores; epilogues are deferred one group into the
    # next nt so the PE never starves the scalar engine's exp stream ----
    vt5 = vt_sb.rearrange("p (t cc i c) -> p t cc i c", t=16, cc=2, i=2, c=128)
    state = {"pend": None, "tail": None}

    def emit_conv(g):
        sl = bass.ts(g, 512)
        ps = psum.tile([128, 512], F32, name=f"cps{g}", tag="sc1")
        for dd in range(NUM_CTX):
            nc.tensor.matmul(ps[:], wf_sb[dd], ctxin_sb[dd][:, sl],
                             start=(dd == 0), stop=(dd == NUM_CTX - 1))
        y = pool.tile([128, 512], F32, name=f"y{g}", tag="y", bufs=2)
        nc.scalar.activation(y[:], ps[:], AF.Identity, bias=bf_sb)
        nc.vector.scalar_tensor_tensor(ctx_sb[:, sl], y[:], 0.1, y[:],
                                       op0=ALU.mult, op1=ALU.max)

    def emit_k(g):
        sl = bass.ts(g, 512)
        ps = psum.tile([128, 512], F32, name=f"kps{g}", tag="pre")
        nc.tensor.matmul(ps[:], wk_sb, ctx_sb[:, sl], start=True, stop=True)
        nc.scalar.activation(kr_sb[:, sl], ps[:], AF.Identity, bias=bk_sb)

    def emit_q(qt):
        sl = bass.ts(qt, 512)
        ps = psum.tile([128, 512], F32, name=f"qps{qt}", tag="sc1")
        for mm in range(2):
            nc.tensor.matmul(ps[:], wq_sb[mm], x_sb[mm][:, sl],
                             start=(mm == 0), stop=(mm == 1))
        nc.scalar.activation(qr_sb[:, sl], ps[:], AF.Identity, bias=bq_sb)

    def emit_vt(g):
        # produce vTFP8 for key chunks j = 4g..4g+3 (pairs 2g, 2g+1)
        for jj in range(JG):
            j = JG * g + jj
            t_pair, ii = j // 2, j % 2
            ps = psum.tile([128, 256], F32, name=f"vps{j}", tag="pre")
            nc.tensor.matmul(ps[:], ctx_sb[:, bass.ts(j, 128)], wv_sb,
                             start=True, stop=True)
            nc.vector.tensor_copy(vt5[:, t_pair, :, ii, :],
                                  ps[:].rearrange("p (cc c) -> p cc c", cc=2))

    def consume():
        if state["pend"] is None:
            return
        gp, h_ps, s32, EA, EB = state["pend"]
        state["pend"] = None
        # h += vT^T @ E via fp8 DoubleRow (contracts 256 keys per matmul)
        for u, Eh in enumerate((EA, EB)):
            t_pair = 2 * gp + u
            rhs = Eh[:, :].rearrange("p (two n) -> p two n", two=2)
            for cc in range(2):
                base = t_pair * 512 + cc * 256
                lhsT = vt_sb[:, base:base + 256].rearrange(
                    "p (two c) -> p two c", two=2)
                nc.tensor.matmul(
                    h_ps[cc][:], lhsT, rhs,
                    start=(t_pair == 0), stop=(t_pair == N // 256 - 1),
                    perf_mode=mybir.MatmulPerfMode.DoubleRow,
                    skip_group_check=True)
        # S32 += ones^T @ E: 4 adjacent col positions run concurrently
        for i in range(JG):
            Eh = (EA, EB)[i // 2]
            nc.tensor.matmul(
                s32[32 * i:32 * (i + 1), :], ones32[:],
                Eh[:, bass.ts(i % 2, NT)],
                start=(gp == 0), stop=(gp == N_JG - 1),
                tile_position=(0, 32 * i), skip_group_check=True)

    def emit_tail():
        if state["tail"] is None:
            return
        nt, h_ps, s32 = state["tail"]
        state["tail"] = None
        qsl = bass.ts(nt, NT)
        # rowsum -> 1/S broadcast
        s32sb = pool.tile([128, NT], BF16, name=f"s32sb{nt}", tag="s32sb", bufs=2)
        nc.vector.tensor_copy(s32sb[:], s32[:])
        sbp = psum.tile([128, NT], F32, name=f"sbp_{nt}", tag="s32")
        nc.tensor.matmul(sbp[:], ones_bc[:], s32sb[:], start=True, stop=True)
        sinv = pool.tile([128, NT], F32, name=f"sinv{nt}", tag="sinv", bufs=2)
        nc.vector.reciprocal_approx_fast(sinv[:], sbp[:])
        # normalize h, output projection, residual, store
        hn = []
        for cc in range(2):
            t = pool.tile([128, NT], BF16, name=f"hn{cc}_{nt}", tag=f"hn{cc}", bufs=2)
            nc.vector.tensor_mul(t[:], h_ps[cc][:], sinv[:])
            hn.append(t)
        for mm in range(2):
            wo_ps = psum.tile([128, NT], F32, name=f"wo{mm}_{nt}", tag=f"h{mm}")
            for kk in range(2):
                nc.tensor.matmul(wo_ps[:], wo_sb[kk][:, bass.ts(mm, 128)], hn[kk][:],
                                 start=(kk == 0), stop=(kk == 1))
            ot = pool.tile([128, NT], F32, name=f"ot{mm}_{nt}", tag=f"ot{mm}", bufs=2)
            nc.vector.scalar_tensor_tensor(ot[:], wo_ps[:], gbo_sb[mm],
                                           x_sb[mm][:, qsl], op0=ALU.add, op1=ALU.add)
            nc.sync.dma_start(d["out"][mm * 128:(mm + 1) * 128, nt * NT:(nt + 1) * NT],
                              ot[:])

    for nt in range(N_NT):
        qsl = bass.ts(nt, NT)
        h_ps = s32 = None
        for g in range(N_JG):
            if nt == 0:
                emit_conv(g)
                emit_k(g)
                if g < N_NT:
                    emit_q(g)
                emit_vt(g)
            Eh2 = []
            for half in range(2):
                sch = psum.tile([128, 2 * NT], F32, name=f"sc{half}_{nt}_{g}",
                                tag=f"sc{half}")
                for ii in range(2):
                    i = half * 2 + ii
                    j = JG * g + i
                    nc.tensor.matmul(
                        sch[:, bass.ts(ii, NT)],
                        kr_sb[32 * i:32 * (i + 1), bass.ts(j, 128)],
                        qr_sb[32 * i:32 * (i + 1), qsl],
                        start=True, stop=True, tile_position=(32 * i, 0),
                        skip_group_check=True)
                E = pool.tile([128, 2 * NT], FP8, name=f"E{half}_{nt}_{g}",
                              tag=f"E{half}", bufs=3)
                nc.scalar.activation(E[:], sch[:], AF.Exp, scale=SCALE)
                Eh2.append(E)
            consume()
            if g == 0:
                # previous nt's epilogue, then THIS nt's accumulators — the
                # allocation order must match the psum tag-ring usage order
                emit_tail()
                h_ps = [psum.tile([128, NT], F32, name=f"h{cc}_{nt}", tag=f"h{cc}")
                        for cc in range(2)]
                s32 = psum.tile([128, NT], F32, name=f"s32_{nt}", tag="s32")
            state["pend"] = (g, h_ps, s32, Eh2[0], Eh2[1])
        state["tail"] = (nt, h_ps, s32)
    consume()
    emit_tail()


def build_program():
    nc = bacc.Bacc("TRN2", debug=False)
    d = {}
    d["ctxin"] = nc.dram_tensor("ctxin", [NUM_CTX * KV_CH, N], BF16,
                                kind="ExternalInput").ap()
    d["xin"] = nc.dram_tensor("xin", [Q_CH, NQ], F32, kind="ExternalInput").ap()
    d["wblob16"] = nc.dram_tensor("wblob16", [128, 1408], BF16,
                                  kind="ExternalInput").ap()
    d["wblob32"] = nc.dram_tensor("wblob32", [128, 261], F32,
                                  kind="ExternalInput").ap()
    d["out"] = nc.dram_tensor("out", [Q_CH, NQ], F32, kind="ExternalOutput").ap()

    with tile.TileContext(nc) as tc:
        with ExitStack() as ctx:
            _emit(nc, tc, ctx, d)
    nc.compile()
    return nc


def make_in_maps(x, context, Wf, bf, Wq, bq, Wk, bk, Wv, bv, Wo, bo, gamma):
    x = np.asarray(x, dtype=np.float32)
    context = np.asarray(context, dtype=np.float32)
    Wf = np.asarray(Wf, dtype=np.float32)
    bf = np.asarray(bf, dtype=np.float32)
    Wq = np.asarray(Wq, dtype=np.float32)
    bq = np.asarray(bq, dtype=np.float32)
    Wk = np.asarray(Wk, dtype=np.float32)
    bk = np.asarray(bk, dtype=np.float32)
    Wv = np.asarray(Wv, dtype=np.float32)
    bv = np.asarray(bv, dtype=np.float32)
    Wo = np.asarray(Wo, dtype=np.float32)
    bo = np.asarray(bo, dtype=np.float32)
    g = float(np.asarray(gamma).reshape(-1)[0])

    wfT = Wf.T                                    # [512, 128] -> 4 chunks
    wkT4 = np.tile(Wk.T, (1, 4))                  # [128, 128]
    wqT4 = np.tile(Wq.T, (1, 4))                  # [256, 128]
    wvT = Wv.T                                    # [128, 256]
    woT = (g * Wo).T                              # [256, 256] -> 2 chunks
    wblob16 = np.concatenate(
        [wfT[dd * 128:(dd + 1) * 128, :] for dd in range(4)]
        + [wkT4, wvT, woT[0:128, :], woT[128:256, :]], axis=1)
    gbo = (g * (Wo @ bv + bo)).reshape(256, 1)
    wblob32 = np.concatenate(
        [wqT4[0:128, :], wqT4[128:256, :], bf.reshape(128, 1),
         np.tile(bk, 4).reshape(128, 1), np.tile(bq, 4).reshape(128, 1),
         gbo[0:128], gbo[128:256]], axis=1)
    shared = {
        "wblob16": np.ascontiguousarray(wblob16).astype(NP_BF16),
        "wblob32": np.ascontiguousarray(wblob32).astype(np.float32),
    }
    xr = x.reshape(B, Q_CH, N)
    ctxr = context.reshape(B, NUM_CTX * KV_CH, N).astype(NP_BF16)
    in_maps = []
    for c in range(N_CORES):
        b, nh = c // 2, c % 2
        m = dict(shared)
        m["ctxin"] = ctxr[b]
        m["xin"] = np.ascontiguousarray(xr[b][:, nh * NQ:(nh + 1) * NQ])
        in_maps.append(m)
    return in_maps


_CACHE = {}


def kernel(**inputs):
    nc = _CACHE.get("nc")
    if nc is None:
        nc = build_program()
        _CACHE["nc"] = nc
    in_maps = make_in_maps(**inputs)
    res = bass_utils.run_bass_kernel_spmd(nc, in_maps, core_ids=list(range(N_CORES)))
    out = np.empty((B, Q_CH, N), dtype=np.float32)
    for c in range(N_CORES):
        b, nh = c // 2, c % 2
        out[b][:, nh * NQ:(nh + 1) * NQ] = res.results[c]["out"]
    return out.reshape(B, Q_CH, H, W)


# revision 13
# speedup vs baseline: 1.0297x; 1.0297x over previous
"""Trainium2 Bass kernel for nn_ChannelFusedCrossAttn.

Reference computation (per batch b, with N = H*W = 4096 spatial positions):
    ctx  = LeakyReLU_0.1(Wf @ context_fused + bf)        # [128, N]
    q    = Wq @ x + bq                                   # [32, N]
    k    = Wk @ ctx + bk                                 # [32, N]
    v    = Wv @ ctx + bv                                 # [256, N]
    attn = softmax(q^T k / sqrt(32), axis=keys)          # [N, N]
    out  = gamma * (Wo @ (v @ attn^T) + bo) + x

Sharding: 8 cores = 4 batches x 2 query-halves of 2048 positions each.
Each core computes ctx/k/v for the full key range of its batch (duplicated
across the pair) plus attention + output projection for its query half.

Device algorithm (per core, n = its 2048 query positions, m = 4096 keys):
  - scores are computed TRANSPOSED (scoreT[m-chunk, n]) so softmax's key-dim
    reduction and the attn@v contraction both keep m on partitions; the
    unnormalized exp() is used directly (scores here are ~N(0, 0.03), so no
    max-subtraction is needed) and the 1/rowsum normalization is applied after
    the v-contraction (division by a per-n scalar commutes with channel
    matmuls).
  - v is built transposed (vT[m, c] = ctx[:,m]^T @ Wv^T) so it can be the
    stationary matmul operand against E[m, n] without any transposes.
  - rowsum S[n] = sum_m E[m, n] rides the tensor engine: column-tiled
    [128,32]-of-ones matmuls (4 concurrent positions) accumulate 32x-replicated
    partial sums which a 1/32-scaled ones matmul then reduces+broadcasts.
  - biases: bq/bk/bf are applied on-chip via per-partition activation bias;
    bv/bo/gamma are folded on the host (gamma*Wo, gamma*(Wo@bv + bo)).
"""

import numpy as np
from contextlib import ExitStack

import concourse.bass as bass
import concourse.bacc as bacc
import concourse.tile as tile
from concourse import mybir
from concourse import bass_utils

F32 = mybir.dt.float32
BF16 = mybir.dt.bfloat16
FP8 = mybir.dt.float8e4
NP_BF16 = mybir.dt.np(BF16)
AF = mybir.ActivationFunctionType
ALU = mybir.AluOpType

# Problem shape (hardcoded per contest contract).
B = 4
Q_CH = 256
KV_CH = 128
NUM_CTX = 4
QK_DIM = 32
H = W = 64
N = H * W            # 4096 keys per batch
N_CORES = 8
NQ = 2048            # query positions per core (N * B / N_CORES)
SCALE = float(QK_DIM) ** -0.5

NT = 512             # n-tile (query) width for the attention inner loop
N_NT = NQ // NT      # 4
JG = 4               # score row-tile group size (concurrent PE row groups)
N_JG = (N // 128) // JG  # 8 j-groups of 4 key-chunks of 128


def _emit(nc, tc, ctx, d):
    """Emit the per-core program. `d` maps dram tensor name -> AP."""
    pool = ctx.enter_context(tc.tile_pool(name="sb", bufs=1))
    psum = ctx.enter_context(tc.tile_pool(name="ps", bufs=1, space="PSUM"))

    # ---- input streams first on the sync HWDGE ring (ctxin quarters so the
    # conv can start early), weights as two packed blobs on the scalar ring ----
    ctxin_sb = []
    for dd in range(NUM_CTX):
        t = pool.tile([128, N], BF16, name=f"ctxin{dd}", tag=f"ctxin{dd}")
        ctxin_sb.append(t)
    for hh in range(4):
        qsl_in = bass.ts(hh, N // 4)
        for dd in range(NUM_CTX):
            nc.sync.dma_start(ctxin_sb[dd][:, qsl_in],
                              d["ctxin"][dd * 128:(dd + 1) * 128, qsl_in])
        if hh == 0:
            x_sb = []
            for mm in range(2):
                t = pool.tile([128, NQ], F32, name=f"x{mm}", tag=f"x{mm}")
                nc.sync.dma_start(t[:], d["xin"][mm * 128:(mm + 1) * 128, :])
                x_sb.append(t)

    wb16 = pool.tile([128, 1408], BF16, tag="wb16")
    nc.scalar.dma_start(wb16[:], d["wblob16"][:, :])
    wb32 = pool.tile([128, 261], F32, tag="wb32")
    nc.scalar.dma_start(wb32[:], d["wblob32"][:, :])
    wf_sb = [wb16[:, dd * 128:(dd + 1) * 128] for dd in range(NUM_CTX)]
    wk_sb = wb16[:, 512:640]
    wv_sb = wb16[:, 640:896]
    wo_sb = [wb16[:, 896 + kk * 256:896 + (kk + 1) * 256] for kk in range(2)]
    wq_sb = [wb32[:, mm * 128:(mm + 1) * 128] for mm in range(2)]
    bf_sb = wb32[:, 256:257]
    bk_sb = wb32[:, 257:258]
    bq_sb = wb32[:, 258:259]
    gbo_sb = [wb32[:, 259 + mm:260 + mm] for mm in range(2)]

    ones32 = pool.tile([128, 32], FP8, tag="ones32")
    nc.gpsimd.memset(ones32[:], 1.0)
    ones_bc = pool.tile([128, 128], BF16, tag="ones_bc")
    nc.gpsimd.memset(ones_bc[:], 1.0 / 32.0)

    ctx_sb = pool.tile([128, N], BF16, tag="ctx")     # fused context, post-LeakyReLU
    kr_sb = pool.tile([128, N], BF16, tag="kr")       # k, 4x-replicated on partitions
    qr_sb = pool.tile([128, NQ], BF16, tag="qr")      # q, 4x-replicated on partitions
    # vT in fp8, pair-interleaved for DoubleRow: offset = t*512 + cc*256 + i*128 + c
    # (t = key-chunk pair, i = pair member, cc = channel chunk, c = channel)
    vt_sb = pool.tile([128, 32 * 256], FP8, tag="vt")
    out_sb = [pool.tile([128, NQ], F32, name=f"o{mm}", tag=f"o{mm}") for mm in range(2)]

    # ---- attention with all producer phases software-pipelined into nt=0:
    # per key-group g, nt0 emits conv(mt=g) -> k(mt=g) -> q(qt=g<4) -> vT(j in g)
    # ahead of that group's scores; epilogues are deferred one group into the
    # next nt so the PE never starves the scalar engine's exp stream ----
    vt5 = vt_sb.rearrange("p (t cc i c) -> p t cc i c", t=16, cc=2, i=2, c=128)
    state = {"pend": None, "tail": None}

    def emit_conv(g):
        sl = bass.ts(g, 512)
        ps = psum.tile([128, 512], F32, name=f"cps{g}", tag="sc1")
        for dd in range(NUM_CTX):
            nc.tensor.matmul(ps[:], wf_sb[dd], ctxin_sb[dd][:, sl],
                             start=(dd == 0), stop=(dd == NUM_CTX - 1))
        y = pool.tile([128, 512], F32, name=f"y{g}", tag="y", bufs=2)
        nc.scalar.activation(y[:], ps[:], AF.Identity, bias=bf_sb)
        nc.vector.scalar_tensor_tensor(ctx_sb[:, sl], y[:], 0.1, y[:],
                                       op0=ALU.mult, op1=ALU.max)

    def emit_k(g):
        sl = bass.ts(g, 512)
        ps = psum.tile([128, 512], F32, name=f"kps{g}", tag="pre")
        nc.tensor.matmul(ps[:], wk_sb, ctx_sb[:, sl], start=True, stop=True)
        nc.scalar.activation(kr_sb[:, sl], ps[:], AF.Identity, bias=bk_sb)

    def emit_q(qt):
        sl = bass.ts(qt, 512)
        ps = psum.tile([128, 512], F32, name=f"qps{qt}", tag="sc1")
        for mm in range(2):
            nc.tensor.matmul(ps[:], wq_sb[mm], x_sb[mm][:, sl],
                             start=(mm == 0), stop=(mm == 1))
        nc.scalar.activation(qr_sb[:, sl], ps[:], AF.Identity, bias=bq_sb)

    def emit_vt(g):
        # produce vTFP8 for key chunks j = 4g..4g+3 (pairs 2g, 2g+1)
        for jj in range(JG):
            j = JG * g + jj
            t_pair, ii = j // 2, j % 2
            ps = psum.tile([128, 256], F32, name=f"vps{j}", tag="pre")
            nc.tensor.matmul(ps[:], ctx_sb[:, bass.ts(j, 128)], wv_sb,
                             start=True, stop=True)
            nc.vector.tensor_copy(vt5[:, t_pair, :, ii, :],
                                  ps[:].rearrange("p (cc c) -> p cc c", cc=2))

    def consume():
        if state["pend"] is None:
            return
        gp, h_ps, s32, EA, EB = state["pend"]
        state["pend"] = None
        # h += vT^T @ E via fp8 DoubleRow (contracts 256 keys per matmul)
        for u, Eh in enumerate((EA, EB)):
            t_pair = 2 * gp + u
            rhs = Eh[:, :].rearrange("p (two n) -> p two n", two=2)
            for cc in range(2):
                base = t_pair * 512 + cc * 256
                lhsT = vt_sb[:, base:base + 256].rearrange(
                    "p (two c) -> p two c", two=2)
                nc.tensor.matmul(
                    h_ps[cc][:], lhsT, rhs,
                    start=(t_pair == 0), stop=(t_pair == N // 256 - 1),
                    perf_mode=mybir.MatmulPerfMode.DoubleRow,
                    skip_group_check=True)
        # S32 += ones^T @ E: 4 adjacent col positions run concurrently
        for i in range(JG):
            Eh = (EA, EB)[i // 2]
            nc.tensor.matmul(
                s32[32 * i:32 * (i + 1), :], ones32[:],
                Eh[:, bass.ts(i % 2, NT)],
                start=(gp == 0), stop=(gp == N_JG - 1),
                tile_position=(0, 32 * i), skip_group_check=True)

    def emit_tail():
        if state["tail"] is None:
            return
        nt, h_ps, s32 = state["tail"]
        state["tail"] = None
        qsl = bass.ts(nt, NT)
        # rowsum -> 1/S broadcast
        s32sb = pool.tile([128, NT], BF16, name=f"s32sb{nt}", tag="s32sb", bufs=2)
        nc.vector.tensor_copy(s32sb[:], s32[:])
        sbp = psum.tile([128, NT], F32, name=f"sbp_{nt}", tag="s32")
        nc.tensor.matmul(sbp[:], ones_bc[:], s32sb[:], start=True, stop=True)
        sinv = pool.tile([128, NT], F32, name=f"sinv{nt}", tag="sinv", bufs=2)
        nc.vector.reciprocal_approx_fast(sinv[:], sbp[:])
        # normalize h, output projection, residual, store
        hn = []
        for cc in range(2):
            t = pool.tile([128, NT], BF16, name=f"hn{cc}_{nt}", tag=f"hn{cc}", bufs=2)
            nc.vector.tensor_mul(t[:], h_ps[cc][:], sinv[:])
            hn.append(t)
        for mm in range(2):
            wo_ps = psum.tile([128, NT], F32, name=f"wo{mm}_{nt}", tag=f"h{mm}")
            for kk in range(2):
                nc.tensor.matmul(wo_ps[:], wo_sb[kk][:, bass.ts(mm, 128)], hn[kk][:],
                                 start=(kk == 0), stop=(kk == 1))
            ot = pool.tile([128, NT], F32, name=f"ot{mm}_{nt}", tag=f"ot{mm}", bufs=2)
            nc.vector.scalar_tensor_tensor(ot[:], wo_ps[:], gbo_sb[mm],
                                           x_sb[mm][:, qsl], op0=ALU.add, op1=ALU.add)
            nc.sync.dma_start(d["out"][mm * 128:(mm + 1) * 128, nt * NT:(nt + 1) * NT],
                              ot[:])

    for nt in range(N_NT):
        qsl = bass.ts(nt, NT)
        h_ps = s32 = None
        if nt == 0:
            # prologue: producers run 2 key-groups ahead of the score stream
            for gp0 in range(2):
                emit_conv(gp0)
                emit_k(gp0)
                emit_q(gp0)
                emit_vt(gp0)
        for g in range(N_JG):
            if nt == 0 and g + 2 < N_JG:
                emit_conv(g + 2)
                emit_k(g + 2)
                if g + 2 < N_NT + 2 and g + 2 >= 2:
                    emit_q(g)
                emit_vt(g + 2)
            Eh2 = []
            for half in range(2):
                sch = psum.tile([128, 2 * NT], F32, name=f"sc{half}_{nt}_{g}",
                                tag=f"sc{half}")
                for ii in range(2):
                    i = half * 2 + ii
                    j = JG * g + i
                    nc.tensor.matmul(
                        sch[:, bass.ts(ii, NT)],
                        kr_sb[32 * i:32 * (i + 1), bass.ts(j, 128)],
                        qr_sb[32 * i:32 * (i + 1), qsl],
                        start=True, stop=True, tile_position=(32 * i, 0),
                        skip_group_check=True)
                E = pool.tile([128, 2 * NT], FP8, name=f"E{half}_{nt}_{g}",
                              tag=f"E{half}", bufs=3)
                nc.scalar.activation(E[:], sch[:], AF.Exp, scale=SCALE)
                Eh2.append(E)
            consume()
            if g == 0:
                # previous nt's epilogue, then THIS nt's accumulators — the
                # allocation order must match the psum tag-ring usage order
                emit_tail()
                h_ps = [psum.tile([128, NT], F32, name=f"h{cc}_{nt}", tag=f"h{cc}")
                        for cc in range(2)]
                s32 = psum.tile([128, NT], F32, name=f"s32_{nt}", tag="s32")
            state["pend"] = (g, h_ps, s32, Eh2[0], Eh2[1])
        state["tail"] = (nt, h_ps, s32)
    consume()
    emit_tail()


def build_program():
    nc = bacc.Bacc("TRN2", debug=False)
    d = {}
    d["ctxin"] = nc.dram_tensor("ctxin", [NUM_CTX * KV_CH, N], BF16,
                                kind="ExternalInput").ap()
    d["xin"] = nc.dram_tensor("xin", [Q_CH, NQ], F32, kind="ExternalInput").ap()
    d["wblob16"] = nc.dram_tensor("wblob16", [128, 1408], BF16,
                                  kind="ExternalInput").ap()
    d["wblob32"] = nc.dram_tensor("wblob32", [128, 261], F32,
                                  kind="ExternalInput").ap()
    d["out"] = nc.dram_tensor("out", [Q_CH, NQ], F32, kind="ExternalOutput").ap()

    with tile.TileContext(nc) as tc:
        with ExitStack() as ctx:
            _emit(nc, tc, ctx, d)
    nc.compile()
    return nc


def make_in_maps(x, context, Wf, bf, Wq, bq, Wk, bk, Wv, bv, Wo, bo, gamma):
    x = np.asarray(x, dtype=np.float32)
    context = np.asarray(context, dtype=np.float32)
    Wf = np.asarray(Wf, dtype=np.float32)
    bf = np.asarray(bf, dtype=np.float32)
    Wq = np.asarray(Wq, dtype=np.float32)
    bq = np.asarray(bq, dtype=np.float32)
    Wk = np.asarray(Wk, dtype=np.float32)
    bk = np.asarray(bk, dtype=np.float32)
    Wv = np.asarray(Wv, dtype=np.float32)
    bv = np.asarray(bv, dtype=np.float32)
    Wo = np.asarray(Wo, dtype=np.float32)
    bo = np.asarray(bo, dtype=np.float32)
    g = float(np.asarray(gamma).reshape(-1)[0])

    wfT = Wf.T                                    # [512, 128] -> 4 chunks
    wkT4 = np.tile(Wk.T, (1, 4))                  # [128, 128]
    wqT4 = np.tile(Wq.T, (1, 4))                  # [256, 128]
    wvT = Wv.T                                    # [128, 256]
    woT = (g * Wo).T                              # [256, 256] -> 2 chunks
    wblob16 = np.concatenate(
        [wfT[dd * 128:(dd + 1) * 128, :] for dd in range(4)]
        + [wkT4, wvT, woT[0:128, :], woT[128:256, :]], axis=1)
    gbo = (g * (Wo @ bv + bo)).reshape(256, 1)
    wblob32 = np.concatenate(
        [wqT4[0:128, :], wqT4[128:256, :], bf.reshape(128, 1),
         np.tile(bk, 4).reshape(128, 1), np.tile(bq, 4).reshape(128, 1),
         gbo[0:128], gbo[128:256]], axis=1)
    shared = {
        "wblob16": np.ascontiguousarray(wblob16).astype(NP_BF16),
        "wblob32": np.ascontiguousarray(wblob32).astype(np.float32),
    }
    xr = x.reshape(B, Q_CH, N)
    ctxr = context.reshape(B, NUM_CTX * KV_CH, N).astype(NP_BF16)
    in_maps = []
    for c in range(N_CORES):
        b, nh = c // 2, c % 2
        m = dict(shared)
        m["ctxin"] = ctxr[b]
        m["xin"] = np.ascontiguousarray(xr[b][:, nh * NQ:(nh + 1) * NQ])
        in_maps.append(m)
    return in_maps


_CACHE = {}


def kernel(**inputs):
    nc = _CACHE.get("nc")
    if nc is None:
        nc = build_program()
        _CACHE["nc"] = nc
    in_maps = make_in_maps(**inputs)
    res = bass_utils.run_bass_kernel_spmd(nc, in_maps, core_ids=list(range(N_CORES)))
    out = np.empty((B, Q_CH, N), dtype=np.float32)
    for c in range(N_CORES):
        b, nh = c // 2, c % 2
        out[b][:, nh * NQ:(nh + 1) * NQ] = res.results[c]["out"]
    return out.reshape(B, Q_CH, H, W)


# revision 14
# speedup vs baseline: 1.0452x; 1.0151x over previous
"""Trainium2 Bass kernel for nn_ChannelFusedCrossAttn.

Reference computation (per batch b, with N = H*W = 4096 spatial positions):
    ctx  = LeakyReLU_0.1(Wf @ context_fused + bf)        # [128, N]
    q    = Wq @ x + bq                                   # [32, N]
    k    = Wk @ ctx + bk                                 # [32, N]
    v    = Wv @ ctx + bv                                 # [256, N]
    attn = softmax(q^T k / sqrt(32), axis=keys)          # [N, N]
    out  = gamma * (Wo @ (v @ attn^T) + bo) + x

Sharding: 8 cores = 4 batches x 2 query-halves of 2048 positions each.
Each core computes ctx/k/v for the full key range of its batch (duplicated
across the pair) plus attention + output projection for its query half.

Device algorithm (per core, n = its 2048 query positions, m = 4096 keys):
  - scores are computed TRANSPOSED (scoreT[m-chunk, n]) so softmax's key-dim
    reduction and the attn@v contraction both keep m on partitions; the
    unnormalized exp() is used directly (scores here are ~N(0, 0.03), so no
    max-subtraction is needed) and the 1/rowsum normalization is applied after
    the v-contraction (division by a per-n scalar commutes with channel
    matmuls).
  - v is built transposed (vT[m, c] = ctx[:,m]^T @ Wv^T) so it can be the
    stationary matmul operand against E[m, n] without any transposes.
  - rowsum S[n] = sum_m E[m, n] rides the tensor engine: column-tiled
    [128,32]-of-ones matmuls (4 concurrent positions) accumulate 32x-replicated
    partial sums which a 1/32-scaled ones matmul then reduces+broadcasts.
  - biases: bq/bk/bf are applied on-chip via per-partition activation bias;
    bv/bo/gamma are folded on the host (gamma*Wo, gamma*(Wo@bv + bo)).
"""

import numpy as np
from contextlib import ExitStack

import concourse.bass as bass
import concourse.bacc as bacc
import concourse.tile as tile
from concourse import mybir
from concourse import bass_utils

F32 = mybir.dt.float32
BF16 = mybir.dt.bfloat16
FP8 = mybir.dt.float8e4
NP_BF16 = mybir.dt.np(BF16)
AF = mybir.ActivationFunctionType
ALU = mybir.AluOpType

# Problem shape (hardcoded per contest contract).
B = 4
Q_CH = 256
KV_CH = 128
NUM_CTX = 4
QK_DIM = 32
H = W = 64
N = H * W            # 4096 keys per batch
N_CORES = 8
NQ = 2048            # query positions per core (N * B / N_CORES)
SCALE = float(QK_DIM) ** -0.5

NT = 512             # n-tile (query) width for the attention inner loop
N_NT = NQ // NT      # 4
JG = 4               # score row-tile group size (concurrent PE row groups)
N_JG = (N // 128) // JG  # 8 j-groups of 4 key-chunks of 128


def _emit(nc, tc, ctx, d):
    """Emit the per-core program. `d` maps dram tensor name -> AP."""
    pool = ctx.enter_context(tc.tile_pool(name="sb", bufs=1))
    psum = ctx.enter_context(tc.tile_pool(name="ps", bufs=1, space="PSUM"))

    # ---- input streams first on the sync HWDGE ring (ctxin quarters so the
    # conv can start early), weights as two packed blobs on the scalar ring ----
    ctxin_sb = []
    for dd in range(NUM_CTX):
        t = pool.tile([128, N], BF16, name=f"ctxin{dd}", tag=f"ctxin{dd}")
        ctxin_sb.append(t)
    for hh in range(4):
        qsl_in = bass.ts(hh, N // 4)
        for dd in range(NUM_CTX):
            nc.sync.dma_start(ctxin_sb[dd][:, qsl_in],
                              d["ctxin"][dd * 128:(dd + 1) * 128, qsl_in])
        if hh == 0:
            x_sb = []
            for mm in range(2):
                t = pool.tile([128, NQ], F32, name=f"x{mm}", tag=f"x{mm}")
                nc.sync.dma_start(t[:], d["xin"][mm * 128:(mm + 1) * 128, :])
                x_sb.append(t)

    wb16 = pool.tile([128, 1408], BF16, tag="wb16")
    nc.scalar.dma_start(wb16[:], d["wblob16"][:, :])
    wb32 = pool.tile([128, 261], F32, tag="wb32")
    nc.scalar.dma_start(wb32[:], d["wblob32"][:, :])
    wf_sb = [wb16[:, dd * 128:(dd + 1) * 128] for dd in range(NUM_CTX)]
    wk_sb = wb16[:, 512:640]
    wv_sb = wb16[:, 640:896]
    wo_sb = [wb16[:, 896 + kk * 256:896 + (kk + 1) * 256] for kk in range(2)]
    wq_sb = [wb32[:, mm * 128:(mm + 1) * 128] for mm in range(2)]
    bf_sb = wb32[:, 256:257]
    bk_sb = wb32[:, 257:258]
    bq_sb = wb32[:, 258:259]
    gbo_sb = [wb32[:, 259 + mm:260 + mm] for mm in range(2)]

    ones32 = pool.tile([128, 32], FP8, tag="ones32")
    nc.gpsimd.memset(ones32[:], 1.0)
    ones_bc = pool.tile([128, 128], BF16, tag="ones_bc")
    nc.gpsimd.memset(ones_bc[:], 1.0 / 32.0)

    ctx_sb = pool.tile([128, N], BF16, tag="ctx")     # fused context, post-LeakyReLU
    kr_sb = pool.tile([128, N], BF16, tag="kr")       # k, 4x-replicated on partitions
    qr_sb = pool.tile([128, NQ], BF16, tag="qr")      # q, 4x-replicated on partitions
    # vT in fp8, pair-interleaved for DoubleRow: offset = t*512 + cc*256 + i*128 + c
    # (t = key-chunk pair, i = pair member, cc = channel chunk, c = channel)
    vt_sb = pool.tile([128, 32 * 256], FP8, tag="vt")
    out_sb = [pool.tile([128, NQ], F32, name=f"o{mm}", tag=f"o{mm}") for mm in range(2)]

    # ---- attention with all producer phases software-pipelined into nt=0:
    # per key-group g, nt0 emits conv(mt=g) -> k(mt=g) -> q(qt=g<4) -> vT(j in g)
    # ahead of that group's scores; epilogues are deferred one group into the
    # next nt so the PE never starves the scalar engine's exp stream ----
    vt5 = vt_sb.rearrange("p (t cc i c) -> p t cc i c", t=16, cc=2, i=2, c=128)
    state = {"pend": None, "tail": None}

    def emit_conv(g):
        sl = bass.ts(g, 512)
        ps = psum.tile([128, 512], F32, name=f"cps{g}", tag="sc1")
        for dd in range(NUM_CTX):
            nc.tensor.matmul(ps[:], wf_sb[dd], ctxin_sb[dd][:, sl],
                             start=(dd == 0), stop=(dd == NUM_CTX - 1))
        y = pool.tile([128, 512], F32, name=f"y{g}", tag="y", bufs=2)
        nc.scalar.activation(y[:], ps[:], AF.Identity, bias=bf_sb)
        nc.vector.scalar_tensor_tensor(ctx_sb[:, sl], y[:], 0.1, y[:],
                                       op0=ALU.mult, op1=ALU.max)

    def emit_k(g):
        sl = bass.ts(g, 512)
        ps = psum.tile([128, 512], F32, name=f"kps{g}", tag="pre")
        nc.tensor.matmul(ps[:], wk_sb, ctx_sb[:, sl], start=True, stop=True)
        nc.scalar.activation(kr_sb[:, sl], ps[:], AF.Identity, bias=bk_sb)

    def emit_q(qt):
        sl = bass.ts(qt, 512)
        ps = psum.tile([128, 512], F32, name=f"qps{qt}", tag="sc1")
        for mm in range(2):
            nc.tensor.matmul(ps[:], wq_sb[mm], x_sb[mm][:, sl],
                             start=(mm == 0), stop=(mm == 1))
        nc.scalar.activation(qr_sb[:, sl], ps[:], AF.Identity, bias=bq_sb)

    def emit_vt(g):
        # produce vTFP8 for key chunks j = 4g..4g+3 (pairs 2g, 2g+1)
        for jj in range(JG):
            j = JG * g + jj
            t_pair, ii = j // 2, j % 2
            ps = psum.tile([128, 256], F32, name=f"vps{j}", tag="pre")
            nc.tensor.matmul(ps[:], ctx_sb[:, bass.ts(j, 128)], wv_sb,
                             start=True, stop=True)
            nc.vector.tensor_copy(vt5[:, t_pair, :, ii, :],
                                  ps[:].rearrange("p (cc c) -> p cc c", cc=2))

    def consume():
        if state["pend"] is None:
            return
        gp, h_ps, s32, EA, EB = state["pend"]
        state["pend"] = None
        # h += vT^T @ E via fp8 DoubleRow (contracts 256 keys per matmul)
        for u, Eh in enumerate((EA, EB)):
            t_pair = 2 * gp + u
            rhs = Eh[:, :].rearrange("p (two n) -> p two n", two=2)
            for cc in range(2):
                base = t_pair * 512 + cc * 256
                lhsT = vt_sb[:, base:base + 256].rearrange(
                    "p (two c) -> p two c", two=2)
                nc.tensor.matmul(
                    h_ps[cc][:], lhsT, rhs,
                    start=(t_pair == 0), stop=(t_pair == N // 256 - 1),
                    perf_mode=mybir.MatmulPerfMode.DoubleRow,
                    skip_group_check=True)
        # S32 += ones^T @ E: 4 adjacent col positions run concurrently
        for i in range(JG):
            Eh = (EA, EB)[i // 2]
            nc.tensor.matmul(
                s32[32 * i:32 * (i + 1), :], ones32[:],
                Eh[:, bass.ts(i % 2, NT)],
                start=(gp == 0), stop=(gp == N_JG - 1),
                tile_position=(0, 32 * i), skip_group_check=True)

    def emit_tail():
        if state["tail"] is None:
            return
        nt, h_ps, s32 = state["tail"]
        state["tail"] = None
        qsl = bass.ts(nt, NT)
        # rowsum -> 1/S broadcast
        s32sb = pool.tile([128, NT], BF16, name=f"s32sb{nt}", tag="s32sb", bufs=2)
        nc.vector.tensor_copy(s32sb[:], s32[:])
        sbp = psum.tile([128, NT], F32, name=f"sbp_{nt}", tag="s32")
        nc.tensor.matmul(sbp[:], ones_bc[:], s32sb[:], start=True, stop=True)
        sinv = pool.tile([128, NT], F32, name=f"sinv{nt}", tag="sinv", bufs=2)
        nc.vector.reciprocal_approx_fast(sinv[:], sbp[:])
        # normalize h, output projection, residual, store
        hn = []
        for cc in range(2):
            t = pool.tile([128, NT], BF16, name=f"hn{cc}_{nt}", tag=f"hn{cc}", bufs=2)
            nc.vector.tensor_mul(t[:], h_ps[cc][:], sinv[:])
            hn.append(t)
        for mm in range(2):
            wo_ps = psum.tile([128, NT], F32, name=f"wo{mm}_{nt}", tag=f"h{mm}")
            for kk in range(2):
                nc.tensor.matmul(wo_ps[:], wo_sb[kk][:, bass.ts(mm, 128)], hn[kk][:],
                                 start=(kk == 0), stop=(kk == 1))
            ot = pool.tile([128, NT], F32, name=f"ot{mm}_{nt}", tag=f"ot{mm}", bufs=2)
            nc.vector.scalar_tensor_tensor(ot[:], wo_ps[:], gbo_sb[mm],
                                           x_sb[mm][:, qsl], op0=ALU.add, op1=ALU.add)
            nc.sync.dma_start(d["out"][mm * 128:(mm + 1) * 128, nt * NT:(nt + 1) * NT],
                              ot[:])

    for nt in range(N_NT):
        qsl = bass.ts(nt, NT)
        h_ps = s32 = None
        if nt == 0:
            # prologue: producers run 2 key-groups ahead of the score stream
            for gp0 in range(2):
                emit_conv(gp0)
                emit_k(gp0)
                emit_q(gp0)
                emit_vt(gp0)
        for g in range(N_JG):
            if nt == 0 and g + 2 < N_JG:
                emit_conv(g + 2)
                emit_k(g + 2)
                if 2 <= g + 2 < N_NT:
                    emit_q(g + 2)
                emit_vt(g + 2)
            Eh2 = []
            for half in range(2):
                sch = psum.tile([128, 2 * NT], F32, name=f"sc{half}_{nt}_{g}",
                                tag=f"sc{half}")
                for ii in range(2):
                    i = half * 2 + ii
                    j = JG * g + i
                    nc.tensor.matmul(
                        sch[:, bass.ts(ii, NT)],
                        kr_sb[32 * i:32 * (i + 1), bass.ts(j, 128)],
                        qr_sb[32 * i:32 * (i + 1), qsl],
                        start=True, stop=True, tile_position=(32 * i, 0),
                        skip_group_check=True)
                E = pool.tile([128, 2 * NT], FP8, name=f"E{half}_{nt}_{g}",
                              tag=f"E{half}", bufs=3)
                nc.scalar.activation(E[:], sch[:], AF.Exp, scale=SCALE)
                Eh2.append(E)
            consume()
            if g == 0:
                # previous nt's epilogue, then THIS nt's accumulators — the
                # allocation order must match the psum tag-ring usage order
                emit_tail()
                h_ps = [psum.tile([128, NT], F32, name=f"h{cc}_{nt}", tag=f"h{cc}")
                        for cc in range(2)]
                s32 = psum.tile([128, NT], F32, name=f"s32_{nt}", tag="s32")
            state["pend"] = (g, h_ps, s32, Eh2[0], Eh2[1])
        state["tail"] = (nt, h_ps, s32)
    consume()
    emit_tail()


def build_program():
    nc = bacc.Bacc("TRN2", debug=False)
    d = {}
    d["ctxin"] = nc.dram_tensor("ctxin", [NUM_CTX * KV_CH, N], BF16,
                                kind="ExternalInput").ap()
    d["xin"] = nc.dram_tensor("xin", [Q_CH, NQ], F32, kind="ExternalInput").ap()
    d["wblob16"] = nc.dram_tensor("wblob16", [128, 1408], BF16,
                                  kind="ExternalInput").ap()
    d["wblob32"] = nc.dram_tensor("wblob32", [128, 261], F32,
                                  kind="ExternalInput").ap()
    d["out"] = nc.dram_tensor("out", [Q_CH, NQ], F32, kind="ExternalOutput").ap()

    with tile.TileContext(nc) as tc:
        with ExitStack() as ctx:
            _emit(nc, tc, ctx, d)
    nc.compile()
    return nc


def make_in_maps(x, context, Wf, bf, Wq, bq, Wk, bk, Wv, bv, Wo, bo, gamma):
    x = np.asarray(x, dtype=np.float32)
    context = np.asarray(context, dtype=np.float32)
    Wf = np.asarray(Wf, dtype=np.float32)
    bf = np.asarray(bf, dtype=np.float32)
    Wq = np.asarray(Wq, dtype=np.float32)
    bq = np.asarray(bq, dtype=np.float32)
    Wk = np.asarray(Wk, dtype=np.float32)
    bk = np.asarray(bk, dtype=np.float32)
    Wv = np.asarray(Wv, dtype=np.float32)
    bv = np.asarray(bv, dtype=np.float32)
    Wo = np.asarray(Wo, dtype=np.float32)
    bo = np.asarray(bo, dtype=np.float32)
    g = float(np.asarray(gamma).reshape(-1)[0])

    wfT = Wf.T                                    # [512, 128] -> 4 chunks
    wkT4 = np.tile(Wk.T, (1, 4))                  # [128, 128]
    wqT4 = np.tile(Wq.T, (1, 4))                  # [256, 128]
    wvT = Wv.T                                    # [128, 256]
    woT = (g * Wo).T                              # [256, 256] -> 2 chunks
    wblob16 = np.concatenate(
        [wfT[dd * 128:(dd + 1) * 128, :] for dd in range(4)]
        + [wkT4, wvT, woT[0:128, :], woT[128:256, :]], axis=1)
    gbo = (g * (Wo @ bv + bo)).reshape(256, 1)
    wblob32 = np.concatenate(
        [wqT4[0:128, :], wqT4[128:256, :], bf.reshape(128, 1),
         np.tile(bk, 4).reshape(128, 1), np.tile(bq, 4).reshape(128, 1),
         gbo[0:128], gbo[128:256]], axis=1)
    shared = {
        "wblob16": np.ascontiguousarray(wblob16).astype(NP_BF16),
        "wblob32": np.ascontiguousarray(wblob32).astype(np.float32),
    }
    xr = x.reshape(B, Q_CH, N)
    ctxr = context.reshape(B, NUM_CTX * KV_CH, N).astype(NP_BF16)
    in_maps = []
    for c in range(N_CORES):
        b, nh = c // 2, c % 2
        m = dict(shared)
        m["ctxin"] = ctxr[b]
        m["xin"] = np.ascontiguousarray(xr[b][:, nh * NQ:(nh + 1) * NQ])
        in_maps.append(m)
    return in_maps


_CACHE = {}


def kernel(**inputs):
    nc = _CACHE.get("nc")
    if nc is None:
        nc = build_program()
        _CACHE["nc"] = nc
    in_maps = make_in_maps(**inputs)
    res = bass_utils.run_bass_kernel_spmd(nc, in_maps, core_ids=list(range(N_CORES)))
    out = np.empty((B, Q_CH, N), dtype=np.float32)
    for c in range(N_CORES):
        b, nh = c // 2, c % 2
        out[b][:, nh * NQ:(nh + 1) * NQ] = res.results[c]["out"]
    return out.reshape(B, Q_CH, H, W)


# revision 15
# speedup vs baseline: 1.0827x; 1.0359x over previous
"""Trainium2 Bass kernel for nn_ChannelFusedCrossAttn.

Reference computation (per batch b, with N = H*W = 4096 spatial positions):
    ctx  = LeakyReLU_0.1(Wf @ context_fused + bf)        # [128, N]
    q    = Wq @ x + bq                                   # [32, N]
    k    = Wk @ ctx + bk                                 # [32, N]
    v    = Wv @ ctx + bv                                 # [256, N]
    attn = softmax(q^T k / sqrt(32), axis=keys)          # [N, N]
    out  = gamma * (Wo @ (v @ attn^T) + bo) + x

Sharding: 8 cores = 4 batches x 2 query-halves of 2048 positions each.
Each core computes ctx/k/v for the full key range of its batch (duplicated
across the pair) plus attention + output projection for its query half.

Device algorithm (per core, n = its 2048 query positions, m = 4096 keys):
  - scores are computed TRANSPOSED (scoreT[m-chunk, n]) so softmax's key-dim
    reduction and the attn@v contraction both keep m on partitions; the
    unnormalized exp() is used directly (scores here are ~N(0, 0.03), so no
    max-subtraction is needed) and the 1/rowsum normalization is applied after
    the v-contraction (division by a per-n scalar commutes with channel
    matmuls).
  - v is built transposed (vT[m, c] = ctx[:,m]^T @ Wv^T) so it can be the
    stationary matmul operand against E[m, n] without any transposes.
  - rowsum S[n] = sum_m E[m, n] rides the tensor engine: column-tiled
    [128,32]-of-ones matmuls (4 concurrent positions) accumulate 32x-replicated
    partial sums which a 1/32-scaled ones matmul then reduces+broadcasts.
  - biases: bq/bk/bf are applied on-chip via per-partition activation bias;
    bv/bo/gamma are folded on the host (gamma*Wo, gamma*(Wo@bv + bo)).
"""

import numpy as np
from contextlib import ExitStack

import concourse.bass as bass
import concourse.bacc as bacc
import concourse.tile as tile
from concourse import mybir
from concourse import bass_utils

F32 = mybir.dt.float32
BF16 = mybir.dt.bfloat16
FP8 = mybir.dt.float8e4
NP_BF16 = mybir.dt.np(BF16)
AF = mybir.ActivationFunctionType
ALU = mybir.AluOpType

# Problem shape (hardcoded per contest contract).
B = 4
Q_CH = 256
KV_CH = 128
NUM_CTX = 4
QK_DIM = 32
H = W = 64
N = H * W            # 4096 keys per batch
N_CORES = 8
NQ = 2048            # query positions per core (N * B / N_CORES)
SCALE = float(QK_DIM) ** -0.5

NT = 512             # n-tile (query) width for the attention inner loop
N_NT = NQ // NT      # 4
JG = 4               # score row-tile group size (concurrent PE row groups)
N_JG = (N // 128) // JG  # 8 j-groups of 4 key-chunks of 128


def _emit(nc, tc, ctx, d):
    """Emit the per-core program. `d` maps dram tensor name -> AP."""
    pool = ctx.enter_context(tc.tile_pool(name="sb", bufs=1))
    psum = ctx.enter_context(tc.tile_pool(name="ps", bufs=1, space="PSUM"))

    # ---- input streams first on the sync HWDGE ring (ctxin quarters so the
    # conv can start early), weights as two packed blobs on the scalar ring ----
    ctxin_sb = []
    for dd in range(NUM_CTX):
        t = pool.tile([128, N], BF16, name=f"ctxin{dd}", tag=f"ctxin{dd}")
        ctxin_sb.append(t)
    for hh in range(4):
        qsl_in = bass.ts(hh, N // 4)
        for dd in range(NUM_CTX):
            nc.sync.dma_start(ctxin_sb[dd][:, qsl_in],
                              d["ctxin"][dd * 128:(dd + 1) * 128, qsl_in])
        if hh == 0:
            x_sb = []
            for mm in range(2):
                t = pool.tile([128, NQ], F32, name=f"x{mm}", tag=f"x{mm}")
                nc.sync.dma_start(t[:], d["xin"][mm * 128:(mm + 1) * 128, :])
                x_sb.append(t)

    wb16 = pool.tile([128, 1408], BF16, tag="wb16")
    nc.scalar.dma_start(wb16[:], d["wblob16"][:, :])
    wb32 = pool.tile([128, 261], F32, tag="wb32")
    nc.scalar.dma_start(wb32[:], d["wblob32"][:, :])
    wf_sb = [wb16[:, dd * 128:(dd + 1) * 128] for dd in range(NUM_CTX)]
    wk_sb = wb16[:, 512:640]
    wv_sb = wb16[:, 640:896]
    wo_sb = [wb16[:, 896 + kk * 256:896 + (kk + 1) * 256] for kk in range(2)]
    wq_sb = [wb32[:, mm * 128:(mm + 1) * 128] for mm in range(2)]
    bf_sb = wb32[:, 256:257]
    bk_sb = wb32[:, 257:258]
    bq_sb = wb32[:, 258:259]
    gbo_sb = [wb32[:, 259 + mm:260 + mm] for mm in range(2)]

    ones32 = pool.tile([128, 32], FP8, tag="ones32")
    nc.gpsimd.memset(ones32[:], 1.0)
    ones_bc = pool.tile([128, 128], BF16, tag="ones_bc")
    nc.gpsimd.memset(ones_bc[:], 1.0 / 32.0)

    ctx_sb = pool.tile([128, N], BF16, tag="ctx")     # fused context, post-LeakyReLU
    kr_sb = pool.tile([128, N], BF16, tag="kr")       # k, 4x-replicated on partitions
    qr_sb = pool.tile([128, NQ], BF16, tag="qr")      # q, 4x-replicated on partitions
    # vT in fp8, pair-interleaved for DoubleRow: offset = t*512 + cc*256 + i*128 + c
    # (t = key-chunk pair, i = pair member, cc = channel chunk, c = channel)
    vt_sb = pool.tile([128, 32 * 256], FP8, tag="vt")
    out_sb = [pool.tile([128, NQ], F32, name=f"o{mm}", tag=f"o{mm}") for mm in range(2)]

    # ---- attention with all producer phases software-pipelined into nt=0:
    # per key-group g, nt0 emits conv(mt=g) -> k(mt=g) -> q(qt=g<4) -> vT(j in g)
    # ahead of that group's scores; epilogues are deferred one group into the
    # next nt so the PE never starves the scalar engine's exp stream ----
    vt5 = vt_sb.rearrange("p (t cc i c) -> p t cc i c", t=16, cc=2, i=2, c=128)
    state = {"pend": None, "tail": None}

    def emit_conv(g):
        sl = bass.ts(g, 512)
        ps = psum.tile([128, 512], F32, name=f"cps{g}", tag="sc1")
        for dd in range(NUM_CTX):
            nc.tensor.matmul(ps[:], wf_sb[dd], ctxin_sb[dd][:, sl],
                             start=(dd == 0), stop=(dd == NUM_CTX - 1))
        y = pool.tile([128, 512], BF16, name=f"y{g}", tag="y", bufs=2)
        nc.vector.tensor_scalar(y[:], ps[:], bf_sb, None, op0=ALU.add)
        nc.vector.scalar_tensor_tensor(ctx_sb[:, sl], y[:], 0.1, y[:],
                                       op0=ALU.mult, op1=ALU.max)

    def emit_k(g):
        sl = bass.ts(g, 512)
        ps = psum.tile([128, 512], F32, name=f"kps{g}", tag="pre")
        nc.tensor.matmul(ps[:], wk_sb, ctx_sb[:, sl], start=True, stop=True)
        nc.scalar.activation(kr_sb[:, sl], ps[:], AF.Identity, bias=bk_sb)

    def emit_q(qt):
        sl = bass.ts(qt, 512)
        ps = psum.tile([128, 512], F32, name=f"qps{qt}", tag="sc1")
        for mm in range(2):
            nc.tensor.matmul(ps[:], wq_sb[mm], x_sb[mm][:, sl],
                             start=(mm == 0), stop=(mm == 1))
        nc.scalar.activation(qr_sb[:, sl], ps[:], AF.Identity, bias=bq_sb)

    def emit_vt(g):
        # produce vTFP8 for key chunks j = 4g..4g+3 (pairs 2g, 2g+1)
        for jj in range(JG):
            j = JG * g + jj
            t_pair, ii = j // 2, j % 2
            ps = psum.tile([128, 256], F32, name=f"vps{j}", tag="pre")
            nc.tensor.matmul(ps[:], ctx_sb[:, bass.ts(j, 128)], wv_sb,
                             start=True, stop=True)
            nc.vector.tensor_copy(vt5[:, t_pair, :, ii, :],
                                  ps[:].rearrange("p (cc c) -> p cc c", cc=2))

    def consume():
        if state["pend"] is None:
            return
        gp, h_ps, s32, EA, EB = state["pend"]
        state["pend"] = None
        # h += vT^T @ E via fp8 DoubleRow (contracts 256 keys per matmul)
        for u, Eh in enumerate((EA, EB)):
            t_pair = 2 * gp + u
            rhs = Eh[:, :].rearrange("p (two n) -> p two n", two=2)
            for cc in range(2):
                base = t_pair * 512 + cc * 256
                lhsT = vt_sb[:, base:base + 256].rearrange(
                    "p (two c) -> p two c", two=2)
                nc.tensor.matmul(
                    h_ps[cc][:], lhsT, rhs,
                    start=(t_pair == 0), stop=(t_pair == N // 256 - 1),
                    perf_mode=mybir.MatmulPerfMode.DoubleRow,
                    skip_group_check=True)
        # S32 += ones^T @ E: 4 adjacent col positions run concurrently
        for i in range(JG):
            Eh = (EA, EB)[i // 2]
            nc.tensor.matmul(
                s32[32 * i:32 * (i + 1), :], ones32[:],
                Eh[:, bass.ts(i % 2, NT)],
                start=(gp == 0), stop=(gp == N_JG - 1),
                tile_position=(0, 32 * i), skip_group_check=True)

    def emit_tail():
        if state["tail"] is None:
            return
        nt, h_ps, s32 = state["tail"]
        state["tail"] = None
        qsl = bass.ts(nt, NT)
        # rowsum -> 1/S broadcast
        s32sb = pool.tile([128, NT], BF16, name=f"s32sb{nt}", tag="s32sb", bufs=2)
        nc.vector.tensor_copy(s32sb[:], s32[:])
        sbp = psum.tile([128, NT], F32, name=f"sbp_{nt}", tag="pre")
        nc.tensor.matmul(sbp[:], ones_bc[:], s32sb[:], start=True, stop=True)
        sinv = pool.tile([128, NT], F32, name=f"sinv{nt}", tag="sinv", bufs=2)
        nc.vector.reciprocal_approx_fast(sinv[:], sbp[:])
        # normalize h, output projection, residual, store
        hn = []
        for cc in range(2):
            t = pool.tile([128, NT], BF16, name=f"hn{cc}_{nt}", tag=f"hn{cc}", bufs=2)
            nc.vector.tensor_mul(t[:], h_ps[cc][:], sinv[:])
            hn.append(t)
        for mm in range(2):
            wo_ps = psum.tile([128, NT], F32, name=f"wo{mm}_{nt}", tag="pre")
            for kk in range(2):
                nc.tensor.matmul(wo_ps[:], wo_sb[kk][:, bass.ts(mm, 128)], hn[kk][:],
                                 start=(kk == 0), stop=(kk == 1))
            ot = pool.tile([128, NT], F32, name=f"ot{mm}_{nt}", tag=f"ot{mm}", bufs=2)
            nc.vector.scalar_tensor_tensor(ot[:], wo_ps[:], gbo_sb[mm],
                                           x_sb[mm][:, qsl], op0=ALU.add, op1=ALU.add)
            nc.sync.dma_start(d["out"][mm * 128:(mm + 1) * 128, nt * NT:(nt + 1) * NT],
                              ot[:])

    for nt in range(N_NT):
        qsl = bass.ts(nt, NT)
        h_ps = s32 = None
        if nt == 0:
            # prologue: producers run 2 key-groups ahead of the score stream
            for gp0 in range(2):
                emit_conv(gp0)
                emit_k(gp0)
                emit_q(gp0)
                emit_vt(gp0)
        for g in range(N_JG):
            if nt == 0 and g + 2 < N_JG:
                emit_conv(g + 2)
                emit_k(g + 2)
                if 2 <= g + 2 < N_NT:
                    emit_q(g + 2)
                emit_vt(g + 2)
            Eh2 = []
            for half in range(2):
                sch = psum.tile([128, 2 * NT], F32, name=f"sc{half}_{nt}_{g}",
                                tag=f"sc{half}")
                for ii in range(2):
                    i = half * 2 + ii
                    j = JG * g + i
                    nc.tensor.matmul(
                        sch[:, bass.ts(ii, NT)],
                        kr_sb[32 * i:32 * (i + 1), bass.ts(j, 128)],
                        qr_sb[32 * i:32 * (i + 1), qsl],
                        start=True, stop=True, tile_position=(32 * i, 0),
                        skip_group_check=True)
                E = pool.tile([128, 2 * NT], FP8, name=f"E{half}_{nt}_{g}",
                              tag=f"E{half}", bufs=3)
                nc.scalar.activation(E[:], sch[:], AF.Exp, scale=SCALE)
                Eh2.append(E)
            if g == 1:
                emit_tail()
            consume()
            if g == 0:
                h_ps = [psum.tile([128, NT], F32, name=f"h{cc}_{nt}", tag=f"h{cc}")
                        for cc in range(2)]
                s32 = psum.tile([128, NT], F32, name=f"s32_{nt}", tag="s32")
            state["pend"] = (g, h_ps, s32, Eh2[0], Eh2[1])
        state["tail"] = (nt, h_ps, s32)
    consume()
    emit_tail()


def build_program():
    nc = bacc.Bacc("TRN2", debug=False)
    d = {}
    d["ctxin"] = nc.dram_tensor("ctxin", [NUM_CTX * KV_CH, N], BF16,
                                kind="ExternalInput").ap()
    d["xin"] = nc.dram_tensor("xin", [Q_CH, NQ], F32, kind="ExternalInput").ap()
    d["wblob16"] = nc.dram_tensor("wblob16", [128, 1408], BF16,
                                  kind="ExternalInput").ap()
    d["wblob32"] = nc.dram_tensor("wblob32", [128, 261], F32,
                                  kind="ExternalInput").ap()
    d["out"] = nc.dram_tensor("out", [Q_CH, NQ], F32, kind="ExternalOutput").ap()

    with tile.TileContext(nc) as tc:
        with ExitStack() as ctx:
            _emit(nc, tc, ctx, d)
    nc.compile()
    return nc


def make_in_maps(x, context, Wf, bf, Wq, bq, Wk, bk, Wv, bv, Wo, bo, gamma):
    x = np.asarray(x, dtype=np.float32)
    context = np.asarray(context, dtype=np.float32)
    Wf = np.asarray(Wf, dtype=np.float32)
    bf = np.asarray(bf, dtype=np.float32)
    Wq = np.asarray(Wq, dtype=np.float32)
    bq = np.asarray(bq, dtype=np.float32)
    Wk = np.asarray(Wk, dtype=np.float32)
    bk = np.asarray(bk, dtype=np.float32)
    Wv = np.asarray(Wv, dtype=np.float32)
    bv = np.asarray(bv, dtype=np.float32)
    Wo = np.asarray(Wo, dtype=np.float32)
    bo = np.asarray(bo, dtype=np.float32)
    g = float(np.asarray(gamma).reshape(-1)[0])

    wfT = Wf.T                                    # [512, 128] -> 4 chunks
    wkT4 = np.tile(Wk.T, (1, 4))                  # [128, 128]
    wqT4 = np.tile(Wq.T, (1, 4))                  # [256, 128]
    wvT = Wv.T                                    # [128, 256]
    woT = (g * Wo).T                              # [256, 256] -> 2 chunks
    wblob16 = np.concatenate(
        [wfT[dd * 128:(dd + 1) * 128, :] for dd in range(4)]
        + [wkT4, wvT, woT[0:128, :], woT[128:256, :]], axis=1)
    gbo = (g * (Wo @ bv + bo)).reshape(256, 1)
    wblob32 = np.concatenate(
        [wqT4[0:128, :], wqT4[128:256, :], bf.reshape(128, 1),
         np.tile(bk, 4).reshape(128, 1), np.tile(bq, 4).reshape(128, 1),
         gbo[0:128], gbo[128:256]], axis=1)
    shared = {
        "wblob16": np.ascontiguousarray(wblob16).astype(NP_BF16),
        "wblob32": np.ascontiguousarray(wblob32).astype(np.float32),
    }
    xr = x.reshape(B, Q_CH, N)
    ctxr = context.reshape(B, NUM_CTX * KV_CH, N).astype(NP_BF16)
    in_maps = []
    for c in range(N_CORES):
        b, nh = c // 2, c % 2
        m = dict(shared)
        m["ctxin"] = ctxr[b]
        m["xin"] = np.ascontiguousarray(xr[b][:, nh * NQ:(nh + 1) * NQ])
        in_maps.append(m)
    return in_maps


_CACHE = {}


def kernel(**inputs):
    nc = _CACHE.get("nc")
    if nc is None:
        nc = build_program()
        _CACHE["nc"] = nc
    in_maps = make_in_maps(**inputs)
    res = bass_utils.run_bass_kernel_spmd(nc, in_maps, core_ids=list(range(N_CORES)))
    out = np.empty((B, Q_CH, N), dtype=np.float32)
    for c in range(N_CORES):
        b, nh = c // 2, c % 2
        out[b][:, nh * NQ:(nh + 1) * NQ] = res.results[c]["out"]
    return out.reshape(B, Q_CH, H, W)
